# revision 6
# baseline (speedup 1.0000x reference)
import sys
import numpy as np

sys.path.insert(0, "/opt/trn_rl_repo")

import concourse.bass as bass  # noqa: E402
import concourse.tile as tile  # noqa: E402
import concourse.mybir as mybir  # noqa: E402
from concourse.bass_utils import run_bass_kernel_spmd  # noqa: E402
from concourse.tile import ScopedClock  # noqa: E402
from contextlib import ExitStack  # noqa: E402

# ---------------------------------------------------------------------------
# Problem constants (hardcoded per contract: kernel.py must be self-contained)
# ---------------------------------------------------------------------------
N_CORES = 8
K = 512          # n_classes
D = 384          # feature dim
GT = 21
THRESH = 500
LEAD = (8, 192, 256)             # features lead shape
N_TOTAL = LEAD[0] * LEAD[1] * LEAD[2]   # 393216 tokens
N_SHARD = N_TOTAL // N_CORES            # 49152 tokens per core
TILES = N_SHARD // 128                  # 384 tiles of 128 tokens
BLK = 32                                # label post-process batching
F32 = mybir.dt.float32
I32 = mybir.dt.int32


# ---------------------------------------------------------------------------
# Walrus workaround: this build's CTRL-class instructions (Drain/NoOp) accept
# only ONE sem wait; Tile's tail drain accumulates one wait per logical
# processor.  Spread them across single-wait NOPs.
# ---------------------------------------------------------------------------
def _patched_drain_and_barrier(self, tick_clock, wait_clock):
    drain_inst = self.nc.sync.drain()
    wait_clock.add_sem_waits(
        drain_inst.ins, ScopedClock({None: tick_clock.global_clock})
    )
    waits = list(drain_inst.ins.sync_info.on_wait)
    if len(waits) > 1:
        drain_inst.ins.sync_info.on_wait = []
        for w in waits:
            nop = self.nc.sync.nop(nofuse=True)
            nop.ins.sync_info = mybir.SyncInfo(on_wait=[w], on_update=[])
    self.nc.all_engine_barrier()
    popped = self.nc._tile_sem_poison_stack.pop()
    assert popped is self._sem_poison
    self.nc.clear_and_free_semaphores(list(self.sems.allocated().values()))
    self.nc.all_engine_barrier()


tile.TileContext._drain_and_barrier = _patched_drain_and_barrier

# Same walrus limitation applies to every instruction class: split any
# instruction carrying >1 sem wait by prepending same-engine NOPs (engines
# execute their queue in order, so an earlier wait on the same engine is
# equivalent).
_orig_add_instruction = tile.TileContext._add_instruction
_wsplit_ctr = [0]


def _patched_add_instruction(self, inst):
    si = getattr(inst, "sync_info", None)
    if si is not None and si.on_wait is not None and len(si.on_wait) > 1:
        waits = list(si.on_wait)
        si.on_wait = [waits[-1]]
        for w in waits[:-1]:
            _wsplit_ctr[0] += 1
            nop = mybir.InstNoOp(
                name=f"wsplit-{_wsplit_ctr[0]}",
                engine=inst.engine,
                ins=[],
                outs=[],
                sync_info=mybir.SyncInfo(on_wait=[w], on_update=[]),
                bass_nofuse=True,
            )
            _orig_add_instruction(self, nop)
    _orig_add_instruction(self, inst)


tile.TileContext._add_instruction = _patched_add_instruction

_CACHED_NC = None


def build_nc():
    global _CACHED_NC
    if _CACHED_NC is not None:
        return _CACHED_NC
    nc = bass.Bass()
    f_in = nc.declare_dram_parameter("f", [N_SHARD, D], F32, isOutput=False)
    ct_in = nc.declare_dram_parameter("ct", [128, 3 * K], F32, isOutput=False)
    pay_in = nc.declare_dram_parameter("payload", [128, K], F32, isOutput=False)
    id_in = nc.declare_dram_parameter("ident", [128, 128], F32, isOutput=False)
    lab_out = nc.declare_dram_parameter("labels_o", [128, TILES], I32, isOutput=True)
    seg_out = nc.declare_dram_parameter("segs_o", [128, TILES], I32, isOutput=True)
    cu_out = nc.declare_dram_parameter("cu_o", [4, 128, D + 1], F32, isOutput=True)

    cu_part = nc.dram_tensor("cu_part", [4, 128, D + 1], F32)
    cu_red = nc.dram_tensor("cu_red", [4, 128, D + 1], F32, addr_space="Shared")

    with tile.TileContext(nc) as tc, ExitStack() as ctx:
        const_p = ctx.enter_context(tc.tile_pool(name="const", bufs=1))
        fpool = ctx.enter_context(tc.tile_pool(name="f", bufs=3))
        ftp = ctx.enter_context(tc.tile_pool(name="ft", bufs=2))
        simsb = ctx.enter_context(tc.tile_pool(name="simsb", bufs=2))
        ohp = ctx.enter_context(tc.tile_pool(name="oh", bufs=2))
        ttp = ctx.enter_context(tc.tile_pool(name="tt", bufs=2))
        tiny = ctx.enter_context(tc.tile_pool(name="tiny", bufs=4))
        blkp = ctx.enter_context(tc.tile_pool(name="blk", bufs=2))
        scrp = ctx.enter_context(tc.tile_pool(name="scr", bufs=2))
        cupool = ctx.enter_context(tc.tile_pool(name="cu", bufs=1, space="PSUM"))
        simps = ctx.enter_context(tc.tile_pool(name="simps", bufs=2, space="PSUM"))
        tpps = ctx.enter_context(tc.tile_pool(name="tpps", bufs=2, space="PSUM"))
        cusb = ctx.enter_context(tc.tile_pool(name="cusb", bufs=1))

        # resident constants
        ct_sb = const_p.tile([128, 3 * K], F32)       # CT chunks side by side
        nc.sync.dma_start(ct_sb[:], ct_in[:])
        pay_sb = const_p.tile([128, K], F32)
        nc.sync.dma_start(pay_sb[:], pay_in[:])
        ident = const_p.tile([128, 128], F32)
        nc.sync.dma_start(ident[:], id_in[:])

        # persistent PSUM accumulators: 4 chunks of [128, 385]
        cu_ps = [
            cupool.tile([128, D + 1], F32, name=f"cu{c}", tag=f"cu{c}")
            for c in range(4)
        ]

        n_blocks = TILES // BLK
        for b in range(n_blocks):
            combo_blk = blkp.tile([128, BLK], F32, tag="combo")
            inv_blk = blkp.tile([128, BLK], F32, tag="inv")
            for j in range(BLK):
                i = b * BLK + j
                # load features tile [128 tokens, 384] (col 384 = ||f|| later)
                f_ext = fpool.tile([128, D + 8], F32, tag="f")
                nc.sync.dma_start(f_ext[:, 0:D], f_in[i * 128:(i + 1) * 128, :])

                # per-token sum of squares (ACT square w/ accumulation)
                sq = scrp.tile([128, D], F32, tag="sq")
                ssq = tiny.tile([128, 1], F32, tag="ssq")
                nc.scalar.activation(
                    sq[:], f_ext[:, 0:D],
                    mybir.ActivationFunctionType.Square,
                    accum_out=ssq[:],
                )
                # n = sqrt(ssq) stored into f_ext col 384 (counts column input)
                nc.scalar.activation(
                    f_ext[:, D:D + 1], ssq[:],
                    mybir.ActivationFunctionType.Sqrt,
                )
                # inv = 1/n
                nc.vector.reciprocal(inv_blk[:, j:j + 1], f_ext[:, D:D + 1])

                # transpose f tile -> fT (PE) then copy PSUM->SBUF (ACT)
                ps_t = tpps.tile([128, D], F32, tag="pst")
                for c in range(3):
                    nc.tensor.matmul(
                        ps_t[:, c * 128:(c + 1) * 128],
                        f_ext[:, c * 128:(c + 1) * 128],
                        ident[:],
                        is_transpose=True,
                        skip_group_check=True,
                    )
                fT = ftp.tile([128, D], F32, tag="ft")
                nc.scalar.copy(fT[:], ps_t[:])

                # sim = fT.T @ CT  -> [128 tokens, 512] accumulated over 3 chunks
                sim_ps = simps.tile([128, K], F32, tag="sim")
                for c in range(3):
                    nc.tensor.matmul(
                        sim_ps[:],
                        fT[:, c * 128:(c + 1) * 128],
                        ct_sb[:, c * K:(c + 1) * K],
                        start=(c == 0),
                        stop=(c == 2),
                        skip_group_check=True,
                    )
                sim_sb = simsb.tile([128, K], F32, tag="simsb")
                nc.scalar.copy(sim_sb[:], sim_ps[:])

                # argmax machinery: rowmax -> scaled onehot -> payload reduce
                maxv = tiny.tile([128, 1], F32, tag="maxv")
                nc.vector.reduce_max(maxv[:], sim_sb[:], axis=mybir.AxisListType.X)
                oh = ohp.tile([128, K], F32, tag="oh")
                nc.vector.tensor_scalar(
                    oh[:], sim_sb[:],
                    maxv[:], inv_blk[:, j:j + 1],
                    mybir.AluOpType.is_equal, mybir.AluOpType.mult,
                )
                tt_scr = ttp.tile([128, K], F32, tag="ttscr")
                nc.vector.tensor_tensor(
                    tt_scr[:], oh[:], pay_sb[:], mybir.AluOpType.mult
                )
                nc.vector.reduce_sum(
                    combo_blk[:, j:j + 1], tt_scr[:],
                    axis=mybir.AxisListType.X,
                )

                # scatter: cu[c] += onehot_scaled[:,c].T @ [f | n]
                last = (i == TILES - 1)
                for c in range(4):
                    nc.tensor.matmul(
                        cu_ps[c][:],
                        oh[:, c * 128:(c + 1) * 128],
                        f_ext[:, 0:D + 1],
                        start=(i == 0),
                        stop=last,
                        skip_group_check=True,
                    )

            # batched label decode for this block of 32 tiles
            nrec_blk = blkp.tile([128, BLK], F32, tag="nrec")
            nc.vector.reciprocal(nrec_blk[:], inv_blk[:])
            combof = blkp.tile([128, BLK], F32, tag="combof")
            nc.vector.tensor_tensor(
                combof[:], combo_blk[:], nrec_blk[:], mybir.AluOpType.mult
            )
            comboq = blkp.tile([128, BLK], F32, tag="comboq")
            nc.vector.tensor_scalar(
                comboq[:], combof[:], 0.25, None, mybir.AluOpType.add
            )
            combo_i = blkp.tile([128, BLK], I32, tag="comboi")
            nc.vector.tensor_copy(combo_i[:], comboq[:])
            lab_i = blkp.tile([128, BLK], I32, tag="labi")
            nc.vector.tensor_scalar(
                lab_i[:], combo_i[:], K - 1, None, mybir.AluOpType.bitwise_and
            )
            seg_i = blkp.tile([128, BLK], I32, tag="segi")
            nc.vector.tensor_scalar(
                seg_i[:], combo_i[:], 9, None,
                mybir.AluOpType.logical_shift_right,
            )
            nc.sync.dma_start(lab_out[:, b * BLK:(b + 1) * BLK], lab_i[:])
            nc.sync.dma_start(seg_out[:, b * BLK:(b + 1) * BLK], seg_i[:])

        # epilogue: move CU accumulators to DRAM, all-reduce, write out
        cu_sb = cusb.tile([128, 4 * (D + 1)], F32)
        for c in range(4):
            nc.scalar.copy(cu_sb[:, c * (D + 1):(c + 1) * (D + 1)], cu_ps[c][:])
        for c in range(4):
            nc.sync.dma_start(
                cu_part[c], cu_sb[:, c * (D + 1):(c + 1) * (D + 1)]
            )
        nc.gpsimd.collective_compute(
            "AllReduce",
            mybir.AluOpType.add,
            replica_groups=[list(range(N_CORES))],
            ins=[cu_part[:]],
            outs=[cu_red[:]],
        )
        nc.sync.dma_start(cu_out[:], cu_red[:])

    _CACHED_NC = nc
    return nc


def _l2norm_np(x, axis=-1, eps=1e-12):
    n = np.linalg.norm(x, axis=axis, keepdims=True)
    return x / np.maximum(n, eps).astype(x.dtype)


def kernel(features, cluster_centers, pseudo_assignment):
    features = np.asarray(features, dtype=np.float32)
    cluster_centers = np.asarray(cluster_centers, dtype=np.float32)
    pseudo = np.asarray(pseudo_assignment)
    lead_shape = features.shape[:-1]

    nc = build_nc()

    # replicated small inputs
    ct_host = (
        cluster_centers.T.reshape(3, 128, K).transpose(1, 0, 2).reshape(128, 3 * K)
    ).astype(np.float32).copy()
    payload_row = (np.arange(K) + K * pseudo.astype(np.int64)).astype(np.float32)
    pay_host = np.broadcast_to(payload_row, (128, K)).copy()
    id_host = np.eye(128, dtype=np.float32)

    f_flat = features.reshape(-1, D)
    in_maps = []
    for core in range(N_CORES):
        shard = f_flat[core * N_SHARD:(core + 1) * N_SHARD]
        in_maps.append({
            "f": np.ascontiguousarray(shard),
            "ct": ct_host,
            "payload": pay_host,
            "ident": id_host,
        })

    res = run_bass_kernel_spmd(nc, in_maps, list(range(N_CORES))).results

    # unshard: labels/segs are [128, TILES] with token = tile*128 + partition
    labels = np.concatenate(
        [res[c]["labels_o"].T.reshape(-1) for c in range(N_CORES)]
    ).astype(np.int32)
    segs = np.concatenate(
        [res[c]["segs_o"].T.reshape(-1) for c in range(N_CORES)]
    )

    # final centroid update from the all-reduced sums/counts (host glue)
    cu_full = res[0]["cu_o"].reshape(512, D + 1)
    cu = cu_full[:, :D].astype(np.float32)
    counts = np.rint(cu_full[:, D]).astype(np.float32)

    center_update = _l2norm_np(cu, axis=1)
    uf = (counts > THRESH).astype(np.float32)[:, None]
    new_centers = _l2norm_np(
        center_update * uf + cluster_centers * (1.0 - uf), axis=1
    ).astype(np.float32)

    pseudo_segs_pred = labels.reshape(lead_shape)
    segs_pred = segs.astype(pseudo.dtype).reshape(lead_shape)
    return pseudo_segs_pred, segs_pred, new_centers


# revision 7
# speedup vs baseline: 1.0022x; 1.0022x over previous
import sys
import numpy as np

sys.path.insert(0, "/opt/trn_rl_repo")

import concourse.bass as bass  # noqa: E402
import concourse.tile as tile  # noqa: E402
import concourse.mybir as mybir  # noqa: E402
from concourse.bass_utils import run_bass_kernel_spmd  # noqa: E402
from concourse.tile import ScopedClock  # noqa: E402
from contextlib import ExitStack  # noqa: E402

# ---------------------------------------------------------------------------
# Problem constants (hardcoded per contract: kernel.py must be self-contained)
# ---------------------------------------------------------------------------
N_CORES = 8
K = 512          # n_classes
D = 384          # feature dim
GT = 21
THRESH = 500
LEAD = (8, 192, 256)             # features lead shape
N_TOTAL = LEAD[0] * LEAD[1] * LEAD[2]   # 393216 tokens
N_SHARD = N_TOTAL // N_CORES            # 49152 tokens per core
TILES = N_SHARD // 128                  # 384 tiles of 128 tokens
BLK = 32                                # label post-process batching
F32 = mybir.dt.float32
I32 = mybir.dt.int32


# ---------------------------------------------------------------------------
# Walrus workaround: this build's CTRL-class instructions (Drain/NoOp) accept
# only ONE sem wait; Tile's tail drain accumulates one wait per logical
# processor.  Spread them across single-wait NOPs.
# ---------------------------------------------------------------------------
def _patched_drain_and_barrier(self, tick_clock, wait_clock):
    drain_inst = self.nc.sync.drain()
    wait_clock.add_sem_waits(
        drain_inst.ins, ScopedClock({None: tick_clock.global_clock})
    )
    waits = list(drain_inst.ins.sync_info.on_wait)
    if len(waits) > 1:
        drain_inst.ins.sync_info.on_wait = []
        for w in waits:
            nop = self.nc.sync.nop(nofuse=True)
            nop.ins.sync_info = mybir.SyncInfo(on_wait=[w], on_update=[])
    self.nc.all_engine_barrier()
    popped = self.nc._tile_sem_poison_stack.pop()
    assert popped is self._sem_poison
    self.nc.clear_and_free_semaphores(list(self.sems.allocated().values()))
    self.nc.all_engine_barrier()


tile.TileContext._drain_and_barrier = _patched_drain_and_barrier

# Same walrus limitation applies to every instruction class: split any
# instruction carrying >1 sem wait by prepending same-engine NOPs (engines
# execute their queue in order, so an earlier wait on the same engine is
# equivalent).
_orig_add_instruction = tile.TileContext._add_instruction
_wsplit_ctr = [0]


def _patched_add_instruction(self, inst):
    si = getattr(inst, "sync_info", None)
    if si is not None and si.on_wait is not None and len(si.on_wait) > 1:
        waits = list(si.on_wait)
        si.on_wait = [waits[-1]]
        for w in waits[:-1]:
            _wsplit_ctr[0] += 1
            nop = mybir.InstNoOp(
                name=f"wsplit-{_wsplit_ctr[0]}",
                engine=inst.engine,
                ins=[],
                outs=[],
                sync_info=mybir.SyncInfo(on_wait=[w], on_update=[]),
                bass_nofuse=True,
            )
            _orig_add_instruction(self, nop)
    _orig_add_instruction(self, inst)


tile.TileContext._add_instruction = _patched_add_instruction

_CACHED_NC = None


def build_nc():
    global _CACHED_NC
    if _CACHED_NC is not None:
        return _CACHED_NC
    nc = bass.Bass()
    f_in = nc.declare_dram_parameter("f", [N_SHARD, D], F32, isOutput=False)
    ct_in = nc.declare_dram_parameter("ct", [128, 3 * K], F32, isOutput=False)
    pay_in = nc.declare_dram_parameter("payload", [128, K], F32, isOutput=False)
    id_in = nc.declare_dram_parameter("ident", [128, 128], F32, isOutput=False)
    lab_out = nc.declare_dram_parameter("labels_o", [128, TILES], I32, isOutput=True)
    seg_out = nc.declare_dram_parameter("segs_o", [128, TILES], I32, isOutput=True)
    cu_out = nc.declare_dram_parameter("cu_o", [4, 128, D + 1], F32, isOutput=True)

    cu_part = nc.dram_tensor("cu_part", [4, 128, D + 1], F32)
    cu_red = nc.dram_tensor("cu_red", [4, 128, D + 1], F32, addr_space="Shared")

    with tile.TileContext(nc) as tc, ExitStack() as ctx:
        const_p = ctx.enter_context(tc.tile_pool(name="const", bufs=1))
        fpool = ctx.enter_context(tc.tile_pool(name="f", bufs=3))
        ftp = ctx.enter_context(tc.tile_pool(name="ft", bufs=2))
        simsb = ctx.enter_context(tc.tile_pool(name="simsb", bufs=2))
        ohp = ctx.enter_context(tc.tile_pool(name="oh", bufs=2))
        ttp = ctx.enter_context(tc.tile_pool(name="tt", bufs=2))
        tiny = ctx.enter_context(tc.tile_pool(name="tiny", bufs=4))
        blkp = ctx.enter_context(tc.tile_pool(name="blk", bufs=2))
        scrp = ctx.enter_context(tc.tile_pool(name="scr", bufs=2))
        cupool = ctx.enter_context(tc.tile_pool(name="cu", bufs=1, space="PSUM"))
        simps = ctx.enter_context(tc.tile_pool(name="simps", bufs=2, space="PSUM"))
        tpps = ctx.enter_context(tc.tile_pool(name="tpps", bufs=2, space="PSUM"))
        cusb = ctx.enter_context(tc.tile_pool(name="cusb", bufs=1))

        # resident constants
        ct_sb = const_p.tile([128, 3 * K], F32)       # CT chunks side by side
        nc.sync.dma_start(ct_sb[:], ct_in[:])
        pay_sb = const_p.tile([128, K], F32)
        nc.sync.dma_start(pay_sb[:], pay_in[:])
        ident = const_p.tile([128, 128], F32)
        nc.sync.dma_start(ident[:], id_in[:])

        # persistent PSUM accumulators: 4 chunks of [128, 385]
        cu_ps = [
            cupool.tile([128, D + 1], F32, name=f"cu{c}", tag=f"cu{c}")
            for c in range(4)
        ]

        n_blocks = TILES // BLK
        for b in range(n_blocks):
            combo_blk = blkp.tile([128, BLK], F32, tag="combo")
            inv_blk = blkp.tile([128, BLK], F32, tag="inv")
            for j in range(BLK):
                i = b * BLK + j
                # load features 4 tiles per DMA (one dma_start per 512 tokens):
                # token = i*128 + p lives at fsuper[p, (i%4)*(D+8) : ... +D]
                if i % 4 == 0:
                    fsuper = fpool.tile([128, 4 * (D + 8)], F32, tag="f")
                    src = f_in[i * 128:(i + 4) * 128, :].rearrange(
                        "(a p) d -> p a d", p=128
                    )
                    dst = fsuper[:].rearrange("p (a w) -> p a w", a=4)[:, :, 0:D]
                    nc.sync.dma_start(dst, src)
                f_ext = fsuper[:, (i % 4) * (D + 8):(i % 4 + 1) * (D + 8)]

                # per-token sum of squares (ACT square w/ accumulation)
                sq = scrp.tile([128, D], F32, tag="sq")
                ssq = tiny.tile([128, 1], F32, tag="ssq")
                nc.scalar.activation(
                    sq[:], f_ext[:, 0:D],
                    mybir.ActivationFunctionType.Square,
                    accum_out=ssq[:],
                )
                # n = sqrt(ssq) stored into f_ext col 384 (counts column input)
                nc.scalar.activation(
                    f_ext[:, D:D + 1], ssq[:],
                    mybir.ActivationFunctionType.Sqrt,
                )
                # inv = 1/n
                nc.vector.reciprocal(inv_blk[:, j:j + 1], f_ext[:, D:D + 1])

                # transpose f tile -> fT (PE) then copy PSUM->SBUF (ACT)
                ps_t = tpps.tile([128, D], F32, tag="pst")
                for c in range(3):
                    nc.tensor.matmul(
                        ps_t[:, c * 128:(c + 1) * 128],
                        f_ext[:, c * 128:(c + 1) * 128],
                        ident[:],
                        is_transpose=True,
                        skip_group_check=True,
                    )
                fT = ftp.tile([128, D], F32, tag="ft")
                nc.scalar.copy(fT[:], ps_t[:])

                # sim = fT.T @ CT  -> [128 tokens, 512] accumulated over 3 chunks
                sim_ps = simps.tile([128, K], F32, tag="sim")
                for c in range(3):
                    nc.tensor.matmul(
                        sim_ps[:],
                        fT[:, c * 128:(c + 1) * 128],
                        ct_sb[:, c * K:(c + 1) * K],
                        start=(c == 0),
                        stop=(c == 2),
                        skip_group_check=True,
                    )
                sim_sb = simsb.tile([128, K], F32, tag="simsb")
                nc.scalar.copy(sim_sb[:], sim_ps[:])

                # argmax machinery: rowmax -> scaled onehot -> payload reduce
                maxv = tiny.tile([128, 1], F32, tag="maxv")
                nc.vector.reduce_max(maxv[:], sim_sb[:], axis=mybir.AxisListType.X)
                oh = ohp.tile([128, K], F32, tag="oh")
                nc.vector.tensor_scalar(
                    oh[:], sim_sb[:],
                    maxv[:], inv_blk[:, j:j + 1],
                    mybir.AluOpType.is_equal, mybir.AluOpType.mult,
                )
                tt_scr = ttp.tile([128, K], F32, tag="ttscr")
                nc.vector.tensor_tensor(
                    tt_scr[:], oh[:], pay_sb[:], mybir.AluOpType.mult
                )
                nc.vector.reduce_sum(
                    combo_blk[:, j:j + 1], tt_scr[:],
                    axis=mybir.AxisListType.X,
                )

                # scatter: cu[c] += onehot_scaled[:,c].T @ [f | n]
                last = (i == TILES - 1)
                for c in range(4):
                    nc.tensor.matmul(
                        cu_ps[c][:],
                        oh[:, c * 128:(c + 1) * 128],
                        f_ext[:, 0:D + 1],
                        start=(i == 0),
                        stop=last,
                        skip_group_check=True,
                    )

            # batched label decode for this block of 32 tiles
            nrec_blk = blkp.tile([128, BLK], F32, tag="nrec")
            nc.vector.reciprocal(nrec_blk[:], inv_blk[:])
            combof = blkp.tile([128, BLK], F32, tag="combof")
            nc.vector.tensor_tensor(
                combof[:], combo_blk[:], nrec_blk[:], mybir.AluOpType.mult
            )
            comboq = blkp.tile([128, BLK], F32, tag="comboq")
            nc.vector.tensor_scalar(
                comboq[:], combof[:], 0.25, None, mybir.AluOpType.add
            )
            combo_i = blkp.tile([128, BLK], I32, tag="comboi")
            nc.vector.tensor_copy(combo_i[:], comboq[:])
            lab_i = blkp.tile([128, BLK], I32, tag="labi")
            nc.vector.tensor_scalar(
                lab_i[:], combo_i[:], K - 1, None, mybir.AluOpType.bitwise_and
            )
            seg_i = blkp.tile([128, BLK], I32, tag="segi")
            nc.vector.tensor_scalar(
                seg_i[:], combo_i[:], 9, None,
                mybir.AluOpType.logical_shift_right,
            )
            nc.sync.dma_start(lab_out[:, b * BLK:(b + 1) * BLK], lab_i[:])
            nc.sync.dma_start(seg_out[:, b * BLK:(b + 1) * BLK], seg_i[:])

        # epilogue: move CU accumulators to DRAM, all-reduce, write out
        cu_sb = cusb.tile([128, 4 * (D + 1)], F32)
        for c in range(4):
            nc.scalar.copy(cu_sb[:, c * (D + 1):(c + 1) * (D + 1)], cu_ps[c][:])
        for c in range(4):
            nc.sync.dma_start(
                cu_part[c], cu_sb[:, c * (D + 1):(c + 1) * (D + 1)]
            )
        nc.gpsimd.collective_compute(
            "AllReduce",
            mybir.AluOpType.add,
            replica_groups=[list(range(N_CORES))],
            ins=[cu_part[:]],
            outs=[cu_red[:]],
        )
        nc.sync.dma_start(cu_out[:], cu_red[:])

    _CACHED_NC = nc
    return nc


def _l2norm_np(x, axis=-1, eps=1e-12):
    n = np.linalg.norm(x, axis=axis, keepdims=True)
    return x / np.maximum(n, eps).astype(x.dtype)


def kernel(features, cluster_centers, pseudo_assignment):
    features = np.asarray(features, dtype=np.float32)
    cluster_centers = np.asarray(cluster_centers, dtype=np.float32)
    pseudo = np.asarray(pseudo_assignment)
    lead_shape = features.shape[:-1]

    nc = build_nc()

    # replicated small inputs
    ct_host = (
        cluster_centers.T.reshape(3, 128, K).transpose(1, 0, 2).reshape(128, 3 * K)
    ).astype(np.float32).copy()
    payload_row = (np.arange(K) + K * pseudo.astype(np.int64)).astype(np.float32)
    pay_host = np.broadcast_to(payload_row, (128, K)).copy()
    id_host = np.eye(128, dtype=np.float32)

    f_flat = features.reshape(-1, D)
    in_maps = []
    for core in range(N_CORES):
        shard = f_flat[core * N_SHARD:(core + 1) * N_SHARD]
        in_maps.append({
            "f": np.ascontiguousarray(shard),
            "ct": ct_host,
            "payload": pay_host,
            "ident": id_host,
        })

    res = run_bass_kernel_spmd(nc, in_maps, list(range(N_CORES))).results

    # unshard: labels/segs are [128, TILES] with token = tile*128 + partition
    labels = np.concatenate(
        [res[c]["labels_o"].T.reshape(-1) for c in range(N_CORES)]
    ).astype(np.int32)
    segs = np.concatenate(
        [res[c]["segs_o"].T.reshape(-1) for c in range(N_CORES)]
    )

    # final centroid update from the all-reduced sums/counts (host glue)
    cu_full = res[0]["cu_o"].reshape(512, D + 1)
    cu = cu_full[:, :D].astype(np.float32)
    counts = np.rint(cu_full[:, D]).astype(np.float32)

    center_update = _l2norm_np(cu, axis=1)
    uf = (counts > THRESH).astype(np.float32)[:, None]
    new_centers = _l2norm_np(
        center_update * uf + cluster_centers * (1.0 - uf), axis=1
    ).astype(np.float32)

    pseudo_segs_pred = labels.reshape(lead_shape)
    segs_pred = segs.astype(pseudo.dtype).reshape(lead_shape)
    return pseudo_segs_pred, segs_pred, new_centers
